# revision 18
# baseline (speedup 1.0000x reference)
"""DistanceTransformLoss on 8 Trainium2 NeuronCores (Bass/Tile).

loss = BCEWithLogits(predictions, targets).mean()
       + sqrt( sum(pen) / max(count(pen != 0), 1) ),
  pen = (sigmoid(pred) > 0.5) * grassfire_dist_H(targets)

Sharding: data-parallel over batch N (32 images -> 4 per core).

Engine split (per core, 32 [128,1024] tile-equivalents), designed from
microbenchmarks so the DVE (which owns the irreducible 2-cycle/elem
grassfire scans) does almost nothing else:
  DVE:    fwd scan u=max(u-1/1024, t_T) from PSUM; rev scan v; plus one
          2x-mode tensor_scalar m=[e>1] (f16) whose accum_out gives sum(m).
  ACT:    e=exp(p) (natural layout), softplus sum via ln(e+1) accum,
          count via Sign(m*v - 2047/2048) accum (counts mv==1 exactly).
  GPSIMD: pt = p*t (natural), mv = m_T*v (transposed) - mult-only engine.
  PE:     t transposes (f32, 2cyc/row), m transposes (f16, 1cyc/row),
          ones-matmul partial sums of pt and mv accumulated in PSUM
          across all 32 iterations (start/stop on first/last).
Host combines in f64:
  bce  = (sum_sp - sum_pt)/N
  sum_pen = 1024*(sum_m - sum_mv);  count = sum_m - (Nc + sum_sign)/2
  loss = bce + sqrt(sum_pen / max(count, 1))
"""
import sys

if "/opt/trn_rl_repo" not in sys.path:
    sys.path.insert(0, "/opt/trn_rl_repo")

import numpy as np
from contextlib import ExitStack

import concourse.bass as bass
import concourse.bacc as bacc
import concourse.tile as tile
from concourse import mybir, masks
from concourse.ap import AP
from concourse.bass_utils import run_bass_kernel_spmd
from concourse.hw_specs import get_activation_tables

N_CORES = 8
N_PER_CORE = 4
H = 1024
W = 1024
WB = W // 128
HB = H // 128
N_ITERS = N_PER_CORE * WB   # 32 transposed tiles per core
N_STRIPES = N_PER_CORE * HB  # 32 natural stripes per core

F32 = mybir.dt.float32
F16 = mybir.dt.float16
BF16 = mybir.dt.bfloat16
# t is read via the bf16 hi-half view of its f32 bits (bf16 truncation of
# f32 is exactly its high 16 bits): {0.0,1.0} -> {0.0,1.0} exactly, so the
# transposes run at 1 cycle/row instead of f32's 2.
TDEC = -1.0 / 1024.0
CNT_BIAS = -2047.0 / 2048.0  # Sign(mv + bias): +1 iff mv==1 (pen==0 & m)

_CACHED_NC = None


def _rev_free(ap):
    (pstep, pcount), (fstep, fcount) = ap.ap[0], ap.ap[1]
    return AP(ap.tensor, ap.offset + (fcount - 1) * fstep,
              [[pstep, pcount], [-fstep, fcount]])


def _build_nc():
    nc = bacc.Bacc("TRN2", target_bir_lowering=False, debug=False,
                   enable_asserts=False)
    t_ext = nc.dram_tensor("targets", [N_PER_CORE, H, W], F32,
                           kind="ExternalInput").ap()
    p_ext = nc.dram_tensor("predictions", [N_PER_CORE, H, W], F32,
                           kind="ExternalInput").ap()
    acc_ext = nc.dram_tensor("acc", [128, 3 * N_ITERS], F32,
                             kind="ExternalOutput").ap()
    acc2_ext = nc.dram_tensor("acc2", [1, 2 * H], F32,
                              kind="ExternalOutput").ap()

    AL = mybir.AluOpType
    AF = mybir.ActivationFunctionType

    with tile.TileContext(nc) as tc, ExitStack() as ctx:
        const_pool = ctx.enter_context(tc.tile_pool(name="const", bufs=1))
        nat_pool = ctx.enter_context(tc.tile_pool(name="nat", bufs=2))
        p_pool = ctx.enter_context(tc.tile_pool(name="p", bufs=3))
        m_pool = ctx.enter_context(tc.tile_pool(name="m", bufs=17))
        e_pool = ctx.enter_context(tc.tile_pool(name="e", bufs=2))
        ptj_pool = ctx.enter_context(tc.tile_pool(name="ptj", bufs=12))
        sc_pool = ctx.enter_context(tc.tile_pool(name="sc", bufs=2))
        mv_pool = ctx.enter_context(tc.tile_pool(name="mv", bufs=4))
        acc_pool = ctx.enter_context(tc.tile_pool(name="acc", bufs=1))
        pst_pool = ctx.enter_context(tc.tile_pool(name="pst", bufs=2, space="PSUM"))
        psm_pool = ctx.enter_context(tc.tile_pool(name="psm", bufs=2, space="PSUM"))
        psa_pool = ctx.enter_context(tc.tile_pool(name="psa", bufs=1, space="PSUM"))

        tables = list(get_activation_tables(nc.m.arch).items())
        set_id = next(i for i, (_, fns) in enumerate(tables)
                      if AF.Exp in fns and AF.Ln in fns and AF.Sign in fns)
        nc.scalar.add_instruction(mybir.InstLoadActFuncSet(
            name=nc.get_next_instruction_name(),
            act_func_set_id=set_id, ins=[], outs=[]))

        idn16 = const_pool.tile([128, 128], F16, tag="idn16")
        masks.make_identity(nc, idn16[:])
        dec = const_pool.tile([128, H], F16, tag="dec")
        nc.gpsimd.memset(dec[:], TDEC)
        ones_col = const_pool.tile([128, 1], F16, tag="ones_col")
        nc.gpsimd.memset(ones_col[:], 1.0)
        idnb = const_pool.tile([128, 128], BF16, tag="idnb")
        masks.make_identity(nc, idnb[:])
        cnt_bias = const_pool.tile([128, 1], F32, tag="cnt_bias")
        nc.gpsimd.memset(cnt_bias[:], CNT_BIAS)

        # accs columns: [0:32]=softplus, [32:64]=sum m, [64:96]=sum sign
        accs = acc_pool.tile([128, 3 * N_ITERS], F32)
        nc.vector.memset(accs[:], 0.0)

        # persistent PSUM accumulators for sum(pt) and sum(mv)
        pt_acc = psa_pool.tile([1, H], F32, tag="pt_acc")
        mv_acc = psa_pool.tile([1, H], F32, tag="mv_acc")

        t_imgs = {}
        m_strs = {}
        ptjs = {}

        def emit_stripe(n, hb):
            it = n * HB + hb
            if hb == 0:
                t_img_new = nat_pool.tile([128, HB * W], F32, tag="t_img")
                t_imgs[n] = t_img_new
                if n == 0:
                    for hb2 in range(HB):
                        nc.sync.dma_start(
                            t_img_new[:, hb2 * W:(hb2 + 1) * W],
                            t_ext[0, hb2 * 128:(hb2 + 1) * 128, :])
            t_img = t_imgs[n]
            m_str = m_pool.tile([128, W], F16, tag="m_str")
            m_strs[(n, hb)] = m_str
            sl = slice(hb * W, (hb + 1) * W)
            if n > 0:
                nc.sync.dma_start(t_img[:, sl],
                                  t_ext[n, hb * 128:(hb + 1) * 128, :])
            p_str = p_pool.tile([128, W], F32, tag="p_str")
            nc.sync.dma_start(p_str[:], p_ext[n, hb * 128:(hb + 1) * 128, :])
            e_str = e_pool.tile([128, W], F16, tag="e_str")
            spj = e_pool.tile([128, W], BF16, tag="spj")
            ptj = ptj_pool.tile([128, W], F16, tag="ptj")
            nc.scalar.activation(e_str[:], p_str[:], AF.Exp)
            nc.scalar.activation(spj[:], e_str[:], AF.Ln, bias=1.0,
                                 accum_out=accs[:, it:it + 1])
            nc.vector.tensor_scalar(
                m_str[:], e_str[:], 1.0, 0.0, AL.is_gt, AL.add,
                accum_out=accs[:, N_ITERS + it:N_ITERS + it + 1])
            nc.gpsimd.tensor_tensor(ptj[:], p_str[:], t_img[:, sl], AL.mult)
            ptjs[(n, hb)] = ptj

        pending_mms = []

        def flush_mm(min_pending):
            while len(pending_mms) > min_pending:
                it2, ptj2, mv2 = pending_mms.pop(0)
                first, last = (it2 == 0), (it2 == N_ITERS - 1)
                for ch in range(2):
                    cs = slice(ch * 512, (ch + 1) * 512)
                    nc.tensor.matmul(pt_acc[:, cs], ones_col[:],
                                     ptj2[:, cs], start=first, stop=last)
                    nc.tensor.matmul(mv_acc[:, cs], ones_col[:], mv2[:, cs],
                                     start=first, stop=last)

        def emit_wb(n, wb):
            it = n * WB + wb
            t_img = t_imgs[n]
            t_hi = t_img[:].bitcast(BF16)[:, 1::2]
            psum_t = pst_pool.tile([128, H], BF16, tag="psum_t")
            psum_m = psm_pool.tile([128, H], F16, tag="psum_m")
            for hb in range(HB):
                off = hb * W + wb * 128
                nc.tensor.transpose(
                    psum_t[:, hb * 128:(hb + 1) * 128],
                    t_hi[:, off:off + 128], idnb[:])
                nc.tensor.transpose(
                    psum_m[:, hb * 128:(hb + 1) * 128],
                    m_strs[(n, hb)][:, wb * 128:(wb + 1) * 128], idn16[:])

            usc = sc_pool.tile([128, H], F16, tag="usc")
            vsc = sc_pool.tile([128, H], F16, tag="vsc")
            m_T = sc_pool.tile([128, H], F16, tag="m_T")
            mv = mv_pool.tile([128, H], F16, tag="mv")
            sgj = sc_pool.tile([128, H], F16, tag="sgj")
            nc.vector.tensor_tensor_scan(
                usc[:], dec[:], psum_t[:], 0.0, AL.add, AL.max)
            nc.vector.tensor_tensor_scan(
                _rev_free(vsc[:]), dec[:], _rev_free(usc[:]), 0.0,
                AL.add, AL.max)
            nc.scalar.activation(m_T[:], psum_m[:], AF.Copy)
            nc.gpsimd.tensor_tensor(mv[:], m_T[:], vsc[:], AL.mult)
            nc.scalar.activation(
                sgj[:], mv[:], AF.Sign, bias=cnt_bias[:],
                accum_out=accs[:, 2 * N_ITERS + it:2 * N_ITERS + it + 1])
            if wb == WB - 1:
                for hb in range(HB):
                    m_strs.pop((n, hb))
            ptj = ptjs.pop((n, wb))
            pending_mms.append((it, ptj, mv))

        # software pipeline: image n+1 stripes interleaved with image n tiles
        for hb in range(HB):
            emit_stripe(0, hb)
        for n in range(N_PER_CORE):
            for k in range(WB):
                if n + 1 < N_PER_CORE:
                    emit_stripe(n + 1, k)
                emit_wb(n, k)
                flush_mm(2)
        flush_mm(0)

        accs2 = acc_pool.tile([1, 2 * H], F32, tag="accs2")
        nc.scalar.activation(accs2[0:1, 0:H], pt_acc[:], AF.Copy)
        nc.scalar.activation(accs2[0:1, H:2 * H], mv_acc[:], AF.Copy)
        nc.sync.dma_start(acc_ext, accs[:])
        nc.sync.dma_start(acc2_ext, accs2[:])

    nc.compile()
    return nc


def _get_nc():
    global _CACHED_NC
    if _CACHED_NC is None:
        _CACHED_NC = _build_nc()
    return _CACHED_NC


def _run(predictions, targets, trace=False, **trace_kwargs):
    p = np.ascontiguousarray(
        np.asarray(predictions, dtype=np.float32).reshape(32, H, W))
    t = np.ascontiguousarray(
        np.asarray(targets, dtype=np.float32).reshape(32, H, W))

    in_maps = []
    for c in range(N_CORES):
        sl = slice(c * N_PER_CORE, (c + 1) * N_PER_CORE)
        in_maps.append({
            "predictions": np.ascontiguousarray(p[sl]),
            "targets": np.ascontiguousarray(t[sl]),
        })

    nc = _get_nc()
    res = run_bass_kernel_spmd(nc, in_maps, list(range(N_CORES)),
                               trace=trace, **trace_kwargs)

    n_core_elems = float(N_PER_CORE * H * W)
    sum_sp = sum_pt = sum_m = sum_mv = sum_cnt = 0.0
    for c in range(N_CORES):
        acc = np.asarray(res.results[c]["acc"], dtype=np.float64)
        acc2 = np.asarray(res.results[c]["acc2"], dtype=np.float64)
        sp = acc[:, 0:N_ITERS].sum()
        m_ = acc[:, N_ITERS:2 * N_ITERS].sum()
        sg = acc[:, 2 * N_ITERS:3 * N_ITERS].sum()
        pt = acc2[0, 0:H].sum()
        mv_ = acc2[0, H:2 * H].sum()
        sum_sp += sp
        sum_pt += pt
        sum_m += m_
        sum_mv += mv_
        sum_cnt += m_ - (n_core_elems + sg) / 2.0

    n_elem = 32.0 * H * W
    bce = (sum_sp - sum_pt) / n_elem
    sum_pen = 1024.0 * (sum_m - sum_mv)
    border = 0.0 if sum_pen <= 0.0 else sum_pen / max(sum_cnt, 1.0)
    loss = bce + np.sqrt(border)
    return np.float32(loss), res


def kernel(predictions, targets):
    loss, _ = _run(predictions, targets)
    return np.asarray(loss, dtype=np.float32)
